# revision 1
# baseline (speedup 1.0000x reference)
"""Trainium2 Bass kernel: Classical STDP weight update.

Math (matches the jax reference with TAU_PLUS == TAU_MINUS, so both
eligibility traces are identical and eff = (A_PLUS - A_MINUS) * trace):

    trace[t, b, p] = sum_{s < t} decay^(t-s) * pre[s, b, p]
    dW[p, q] = (A+ - A-)/(B*T) * sum_{t, b} trace[t,b,p] * post[t,b,q]

The trace is a lower-triangular Toeplitz matmul per batch element:
trace_b = L @ pre_b with L[t, s] = decay^(t-s) (t > s).  So per batch
element b the whole computation is two chained TensorE matmuls:

    stage 1: trace_b [T, 256]  = (c*L^T).T @ pre_b        (c folded into L)
    stage 2: dW_h   [128, 256] += trace_b[:, h*128:...].T @ post_b

Sharding: data-parallel over the batch (512 -> 64 per core on 8 cores);
each core emits a partial dW [256, 256]; the 8 partials are summed on
the host (the /(B*T) mean commutes with the reduction).

Spikes are 0/1 so bf16 inputs are lossless and halve HBM traffic.
PSUM accumulates in fp32 throughout.

Sync-wait budget note: this toolchain's walrus rejects any compute/DMA
instruction carrying more than ONE semaphore wait, and the final
all-engine drain gets one wait per DMA lane + engine used.  Hence:
  * pre, post and L^T are packed into a single DRAM tensor so each DMA
    group is ONE dma_start (one lane, one sem tick).  After stage 1
    waits on that lane, the PE's vector clock covers the post data too,
    so stage-2 matmuls only ever wait on the DVE trace copy.
  * dedicated SBUF buffers (no recycling) keep input DMAs at zero waits.
  * the trace-PSUM pool has 5 bufs so a stage-1 matmul's slot-reuse WAR
    is covered by a DVE tick the PE has already observed.
"""

import numpy as np
import ml_dtypes

# Problem constants (hardcoded per the harness contract).
B, T, N_PRE, N_POST = 512, 100, 256, 256
N_CORES = 8
B_SHARD = B // N_CORES  # 64
A_PLUS, A_MINUS = 0.005, 0.00525
TAU_PLUS = 20.0
DT_ = 1.0

GROUP = 8    # batch elements per input DMA group (8 groups)
CHUNK = 8    # batch elements per compute chunk (trace tile / stage-2 unit)
SUB = 2      # batch elements per stage-1 matmul (2*256 = 512 cols = 1 bank)

N_GROUPS = B_SHARD // GROUP
N_ROWS = 2 * B_SHARD  # pre/post rows in the packed fp8 tensor

_NC_CACHE = {}


def _lt_matrix() -> np.ndarray:
    """c * L^T as f32: LT[s, t] = (A+ - A-) * decay^(t-s) for t > s else 0."""
    decay = np.exp(np.float64(-DT_ / TAU_PLUS))
    idx = np.arange(T)
    diff = idx[None, :] - idx[:, None]  # t - s
    lt = np.where(diff > 0, (A_PLUS - A_MINUS) * decay ** diff, 0.0)
    return lt.astype(np.float32)


def _build(repeat=1, split=True):
    """Build the per-core Bass program (shard of 64 batch elements).

    repeat>1 unrolls the whole body N times inside one NEFF (bench only:
    wall-clock slope over N isolates device time from dispatch overhead).
    """
    import concourse.bass as bass
    import concourse.tile as tile
    from concourse import mybir

    f32 = mybir.dt.float32
    bf16 = mybir.dt.bfloat16
    f8 = mybir.dt.float8e4
    ds = bass.ds

    nc = bass.Bass()
    # Packed fp8 input, t-major ([T, rows, P]) so each DMA partition row is
    # one long contiguous read (spikes are 0/1 so fp8e4 is lossless and
    # halves HBM traffic again).
    data_d = nc.declare_dram_parameter("data", [T, N_ROWS, N_PRE], f8, isOutput=False)
    lt_d = nc.declare_dram_parameter("lt", [T, T], bf16, isOutput=False)
    dw_d = nc.declare_dram_parameter("dw", [N_PRE, N_POST], f32, isOutput=True)

    chunks_per_group = GROUP // CHUNK
    n_chunks = B_SHARD // CHUNK
    n_sub = CHUNK // SUB

    with tile.TileContext(nc) as tc:
        with (
            tc.tile_pool(name="const", bufs=1) as cpool,
            tc.tile_pool(name="io", bufs=1) as io_pool,
            tc.tile_pool(name="tr", bufs=n_chunks) as tr_pool,
            tc.tile_pool(name="psum", bufs=5, space="PSUM") as ps_pool,
            tc.tile_pool(name="acc", bufs=1, space="PSUM") as acc_pool,
        ):
            # dW accumulators: one PSUM bank per 128-row half of dW.
            dw_ps = [
                acc_pool.tile([128, N_POST], f32, tag=f"dw{h}", name=f"dw_ps{h}")
                for h in range(2)
            ]

            def stage2(trace_t, post_rows, first, last):
                for bi in range(CHUNK):
                    for h in range(2):
                        nc.tensor.matmul(
                            dw_ps[h][:],
                            trace_t[:, bi, ds(h * 128, 128)],
                            post_rows[:, bi, :],
                            start=first and bi == 0,
                            stop=last and bi == CHUNK - 1,
                            skip_group_check=True,
                        )

            def emit_body():
                lt_tile = cpool.tile([T, T], bf16, tag="lt", name="lt_tile")
                nc.sync.dma_start(lt_tile[:], lt_d[:])
                lt_t = lt_tile[:]
                pending = []  # (trace_t, post_rows)
                emitted = 0
                chunk_i = 0
                for g in range(N_GROUPS):
                    lo = 2 * GROUP * g
                    grp_t = io_pool.tile([T, 2 * GROUP, N_PRE], f8, tag=f"grp{g}",
                                         name=f"grp_t{g}")
                    if g == 0:
                        # Split group 0 so stage 1 starts after just the pre
                        # half of the first transfer has landed.
                        nc.sync.dma_start(
                            grp_t[:, 0:GROUP, :], data_d[:, lo : lo + GROUP, :])
                        nc.sync.dma_start(
                            grp_t[:, GROUP : 2 * GROUP, :],
                            data_d[:, lo + GROUP : lo + 2 * GROUP, :])
                    else:
                        nc.sync.dma_start(
                            grp_t[:], data_d[:, lo : lo + 2 * GROUP, :])
                    off = 0
                    for cc in range(chunks_per_group):
                        pbase = cc * CHUNK
                        trace_t = tr_pool.tile([T, CHUNK, N_PRE], bf16, tag="trace")
                        for j in range(n_sub):
                            tr_ps = ps_pool.tile([T, SUB, N_PRE], f32, tag="trps")
                            nc.tensor.matmul(
                                tr_ps[:],
                                lt_t,
                                grp_t[:, ds(off + pbase + j * SUB, SUB), :],
                                start=True,
                                stop=True,
                                skip_group_check=True,
                            )
                            # Alternate copy engine per sub-block: copies of
                            # one chunk run on both engines concurrently, so
                            # they finish before the next chunk's stage-1
                            # matmuls do (PE never stalls on the trace).
                            if j % 2 == 0:
                                nc.vector.tensor_copy(
                                    trace_t[:, ds(j * SUB, SUB), :], tr_ps[:]
                                )
                            else:
                                nc.scalar.copy(
                                    trace_t[:, ds(j * SUB, SUB), :], tr_ps[:]
                                )
                        chunk_i += 1
                        post_rows = grp_t[:, ds(off + GROUP + pbase, CHUNK), :]
                        pending.append((trace_t, post_rows))
                        # Skew stage 2 one chunk behind so PE never stalls on
                        # the copy of the chunk it just produced.
                        if len(pending) >= 2:
                            args = pending.pop(0)
                            stage2(*args, first=(emitted == 0), last=False)
                            emitted += 1
                args = pending.pop(0)
                stage2(*args, first=(emitted == 0), last=True)

                # Bounce PSUM -> SBUF, then one store via SWDGE (gpsimd): its
                # queue has no prior traffic, so the DMA needs one sync wait.
                out_sb = cpool.tile([128, 2, N_POST], f32, tag="osb",
                                    name="out_sb")
                for h in range(2):
                    nc.vector.tensor_copy(out_sb[:, h, :], dw_ps[h][:])
                nc.gpsimd.dma_start(
                    dw_d[:].rearrange("(h p) q -> p h q", h=2), out_sb[:]
                )

            for _rep in range(repeat):
                emit_body()

    if split:
        _split_multiwaits(nc)
    return nc


def _split_multiwaits(nc):
    """Walrus on this toolchain allows one sync wait per instruction; hoist
    extra waits onto single-wait NOPs preceding the instruction (sequential
    sem-ge waits are equivalent to a combined wait)."""
    from concourse import mybir

    for fn in nc.m.functions:
        for bb in fn.blocks:
            out = []
            changed = False
            for inst in bb.instructions:
                si = inst.sync_info
                waits = list(si.on_wait) if (si is not None and si.on_wait) else []
                if len(waits) > 1:
                    changed = True
                    for w in waits[:-1]:
                        out.append(mybir.InstNoOp(
                            name=nc.get_next_instruction_name(),
                            ins=[], outs=[],
                            sync_info=mybir.SyncInfo(on_wait=[w], on_update=[]),
                            bass_nofuse=True,
                            engine=inst.engine,
                        ))
                    si.on_wait = waits[-1:]
                out.append(inst)
            if changed:
                bb.instructions = out


def _get_nc():
    if "nc" not in _NC_CACHE:
        _NC_CACHE["nc"] = _build()
    return _NC_CACHE["nc"]


def _pack_core(pre_b, post_b):
    """Build the packed t-major [T, N_ROWS, N_PRE] fp8 tensor for one core."""
    out = np.zeros((T, N_ROWS, N_PRE), dtype=ml_dtypes.float8_e4m3)
    for g in range(N_GROUPS):
        lo = 2 * GROUP * g
        sl = slice(g * GROUP, (g + 1) * GROUP)
        out[:, lo : lo + GROUP] = pre_b[sl].transpose(1, 0, 2)
        out[:, lo + GROUP : lo + 2 * GROUP] = post_b[sl].transpose(1, 0, 2)
    return out


def _make_in_maps(pre_spikes, post_spikes):
    pre = np.asarray(pre_spikes, dtype=np.float32).astype(ml_dtypes.float8_e4m3)
    post = np.asarray(post_spikes, dtype=np.float32).astype(ml_dtypes.float8_e4m3)
    lt = _lt_matrix().astype(ml_dtypes.bfloat16)
    return [
        {
            "data": _pack_core(
                pre[i * B_SHARD : (i + 1) * B_SHARD],
                post[i * B_SHARD : (i + 1) * B_SHARD],
            ),
            "lt": np.ascontiguousarray(lt),
        }
        for i in range(N_CORES)
    ]


def kernel(pre_spikes, post_spikes, weights=None, **unused):
    from concourse.bass_utils import run_bass_kernel_spmd

    nc = _get_nc()
    in_maps = _make_in_maps(pre_spikes, post_spikes)
    res = run_bass_kernel_spmd(nc, in_maps, core_ids=list(range(N_CORES)))
    partial = np.stack([r["dw"] for r in res.results])  # [8, 256, 256] f32
    dw = partial.sum(axis=0) / np.float32(B * T)
    return dw.astype(np.float32)



# revision 4
# speedup vs baseline: 1.3832x; 1.3832x over previous
"""Trainium2 Bass kernel: Classical STDP weight update (fp8 DoubleRow).

Math (TAU_PLUS == TAU_MINUS, so both traces are equal and
eff = (A+ - A-) * trace):

    trace[t, b, p] = sum_{s < t} decay^(t-s) * pre[s, b, p]
    dW[p, q] = (A+ - A-)/(B*T) * sum_{t, b} trace[t,b,p] * post[t,b,q]

Sharding: data-parallel over batch (512 -> 64 per core on 8 cores); each
core emits a partial dW; the partials are summed and scaled on the host
(the mean commutes with the reduction).

Per-core pipeline, everything in fp8e4m3 (spikes are 0/1 so the inputs
are lossless; the trace is ~O(1..20) so fp8 costs ~3% relative noise per
value, which averages out far below the 2e-2 gate):

  stage 1 (one DoubleRow matmul per b):
      trace_psum[tp=128, p=256] = LT8[sk=50, si=2, tp=128] (.) pre8[sk, si, p]
    The contraction s = si*50 + sk packs 100 timesteps into 50 partitions
    x 2 fp8 k-tiles (DoubleRow = 0.5 cycles per output column).  The
    stationary free dim must be exactly 128 (ISA dual-fp8 restriction),
    so t is zero-padded 100->128 via zero columns in LT8.

  trace copy: PSUM f32 -> SBUF fp8, 4-b chunks, round-robined across
    DVE / Act / Pool so the three engines drain ~17us of copy work in
    ~7us wall while the DMA stream is still running.

  stage 2 (two DoubleRow matmuls per b-PAIR):
      dw_psum[h][m=128, q=256] += trace8[t=100, bi=2, h*128+m] (.) post8[t, bi, q]
    The k-tile dim bi packs TWO batch elements per matmul (contraction
    t x 2b = 200 <= 256), halving stage-2 matmul count.

  The post DMA is split [16,16,16,14,2] b so the last transfer is tiny
  and the tail after the final input is just one pair of matmuls, the
  accumulator copies, and the bf16 store.

Host applies (A+ - A-)/(B*T) and the cross-core sum.
"""

import numpy as np
import ml_dtypes

# Problem constants (hardcoded per the harness contract).
B, T, N_PRE, N_POST = 512, 100, 256, 256
N_CORES = 8
B_SHARD = B // N_CORES  # 64
A_PLUS, A_MINUS = 0.005, 0.00525
TAU_PLUS = 20.0
DT_ = 1.0

TP = 128          # t padded to the 128-column dual-fp8 stationary size
SK = 50           # contraction partitions for stage 1 (s = si*50 + sk)
CHUNK = 4         # b per trace-copy chunk
N_CHUNKS = B_SHARD // CHUNK       # 16
N_PAIRS = B_SHARD // 2            # 32
PRE_BLOCK = 16                    # b per pre DMA
POST_SPLIT = (16, 16, 16, 14, 2)  # b per post DMA

# chunk -> copy engine: A=Activation(scalar), D=DVE(vector).
# (GPSIMD cannot read PSUM, so Pool can't help with trace copies.)
# Act is slightly faster per copy, so it takes 9 of 16.
COPY_SCHED = "ADAADADAADADAADA"

_NC_CACHE = {}


def _lt_matrix() -> np.ndarray:
    """LT8[sk, si, t] = decay^(t-s) for s < t < T else 0, s = si*50 + sk."""
    decay = np.exp(np.float64(-DT_ / TAU_PLUS))
    s = np.arange(T).reshape(2, SK)  # [si, sk]
    t = np.arange(TP)
    diff = t[None, None, :] - s[:, :, None]  # [si, sk, t]
    lt = np.where((diff > 0) & (t[None, None, :] < T), decay ** diff, 0.0)
    return np.ascontiguousarray(lt.transpose(1, 0, 2)).astype(
        ml_dtypes.float8_e4m3
    )


def _build():
    import concourse.bass as bass
    import concourse.tile as tile
    from concourse import mybir

    f32 = mybir.dt.float32
    bf16 = mybir.dt.bfloat16
    f8 = mybir.dt.float8e4
    DR = mybir.MatmulPerfMode.DoubleRow
    ds = bass.ds

    nc = bass.Bass()
    lt_d = nc.declare_dram_parameter("lt", [SK, 2, TP], f8, isOutput=False)
    pre_d = nc.declare_dram_parameter(
        "pre", [SK, B_SHARD, 2, N_PRE], f8, isOutput=False
    )
    post_d = nc.declare_dram_parameter(
        "post", [T, B_SHARD, N_POST], f8, isOutput=False
    )
    dw_d = nc.declare_dram_parameter("dw", [128, 2, N_POST], bf16, isOutput=True)

    with tile.TileContext(nc) as tc:
        with (
            tc.tile_pool(name="io", bufs=1) as io_pool,
            tc.tile_pool(name="tr", bufs=1) as tr_pool,
            tc.tile_pool(name="psum", bufs=3, space="PSUM") as ps_pool,
            tc.tile_pool(name="acc", bufs=1, space="PSUM") as acc_pool,
        ):
            lt_t = io_pool.tile([SK, 2, TP], f8, tag="lt", name="lt_t")
            pre_t = io_pool.tile(
                [SK, B_SHARD, 2, N_PRE], f8, tag="pre", name="pre_t"
            )
            post_t = io_pool.tile(
                [T, B_SHARD, N_POST], f8, tag="post", name="post_t"
            )
            trace8 = tr_pool.tile(
                [T, B_SHARD, N_PRE], f8, tag="tr8", name="trace8"
            )

            # Input stream: lt, 4x16b pre, then post in [16,16,16,14,2].
            nc.sync.dma_start(lt_t[:], lt_d[:])
            for g in range(B_SHARD // PRE_BLOCK):
                lo = g * PRE_BLOCK
                nc.sync.dma_start(
                    pre_t[:, ds(lo, PRE_BLOCK), :, :],
                    pre_d[:, ds(lo, PRE_BLOCK), :, :],
                )
            lo = 0
            for n in POST_SPLIT:
                nc.sync.dma_start(
                    post_t[:, ds(lo, n), :], post_d[:, ds(lo, n), :]
                )
                lo += n

            # dW accumulators, one PSUM tile per 128-row half.
            dw_ps = [
                acc_pool.tile([128, N_POST], f32, tag=f"dw{h}", name=f"dw_ps{h}")
                for h in range(2)
            ]

            # Stage 1 + trace copies.
            for c in range(N_CHUNKS):
                tr_ps = ps_pool.tile([TP, CHUNK, N_PRE], f32, tag="trps",
                                     name="tr_ps")
                for j in range(CHUNK):
                    nc.tensor.matmul(
                        tr_ps[:, j, :],
                        lt_t[:],
                        pre_t[:, c * CHUNK + j, :, :],
                        start=True,
                        stop=True,
                        perf_mode=DR,
                        skip_group_check=True,
                    )
                eng = COPY_SCHED[c]
                dst = trace8[:, ds(c * CHUNK, CHUNK), :]
                src = tr_ps[0:T, :, :]
                if eng == "D":
                    nc.vector.tensor_copy(dst, src)
                else:
                    nc.scalar.copy(dst, src)

            # Stage 2: one b-pair per pair of half-matmuls.
            for p in range(N_PAIRS):
                for h in range(2):
                    nc.tensor.matmul(
                        dw_ps[h][:],
                        trace8[:, ds(2 * p, 2), ds(h * 128, 128)],
                        post_t[:, ds(2 * p, 2), :],
                        start=(p == 0),
                        stop=(p == N_PAIRS - 1),
                        perf_mode=DR,
                        skip_group_check=True,
                    )

            # Accumulators -> SBUF bf16 -> one store.
            out_sb = io_pool.tile([128, 2, N_POST], bf16, tag="osb",
                                  name="out_sb")
            nc.vector.tensor_copy(out_sb[:, 0, :], dw_ps[0][:])
            nc.scalar.copy(out_sb[:, 1, :], dw_ps[1][:])
            nc.sync.dma_start(dw_d[:], out_sb[:])

    _split_multiwaits(nc)
    return nc


def _split_multiwaits(nc):
    """Walrus on this toolchain allows one sync wait per instruction; hoist
    extra waits onto single-wait NOPs preceding the instruction (sequential
    sem-ge waits are equivalent to a combined wait)."""
    from concourse import mybir

    for fn in nc.m.functions:
        for bb in fn.blocks:
            out = []
            changed = False
            for inst in bb.instructions:
                si = inst.sync_info
                waits = list(si.on_wait) if (si is not None and si.on_wait) else []
                if len(waits) > 1:
                    changed = True
                    for w in waits[:-1]:
                        out.append(mybir.InstNoOp(
                            name=nc.get_next_instruction_name(),
                            ins=[], outs=[],
                            sync_info=mybir.SyncInfo(on_wait=[w], on_update=[]),
                            bass_nofuse=True,
                            engine=inst.engine,
                        ))
                    si.on_wait = waits[-1:]
                out.append(inst)
            if changed:
                bb.instructions = out


def _get_nc():
    if "nc" not in _NC_CACHE:
        _NC_CACHE["nc"] = _build()
    return _NC_CACHE["nc"]


def _make_in_maps(pre_spikes, post_spikes):
    f8np = ml_dtypes.float8_e4m3
    pre = np.asarray(pre_spikes, dtype=np.float32)
    post = np.asarray(post_spikes, dtype=np.float32)
    # pre8[core, sk, b, si, p] = pre[core*64+b, si*50+sk, p]
    pre8 = np.ascontiguousarray(
        pre.reshape(N_CORES, B_SHARD, 2, SK, N_PRE).transpose(0, 3, 1, 2, 4)
    ).astype(f8np)
    # post8[core, t, b, q] = post[core*64+b, t, q]
    post8 = np.ascontiguousarray(
        post.reshape(N_CORES, B_SHARD, T, N_POST).transpose(0, 2, 1, 3)
    ).astype(f8np)
    lt = _lt_matrix()
    return [
        {"lt": lt, "pre": pre8[i], "post": post8[i]}
        for i in range(N_CORES)
    ]


def kernel(pre_spikes, post_spikes, weights=None, **unused):
    from concourse.bass_utils import run_bass_kernel_spmd

    nc = _get_nc()
    in_maps = _make_in_maps(pre_spikes, post_spikes)
    res = run_bass_kernel_spmd(nc, in_maps, core_ids=list(range(N_CORES)))
    # dw partials: [core][128, 2, 256] bf16 -> [256, 256] f32
    partial = np.stack(
        [np.asarray(r["dw"], dtype=np.float32) for r in res.results]
    )  # [8, 128, 2, 256]
    dw = partial.sum(axis=0).transpose(1, 0, 2).reshape(N_PRE, N_POST)
    scale = np.float32((A_PLUS - A_MINUS) / (B * T))
    return (dw * scale).astype(np.float32)
